# revision 41
# baseline (speedup 1.0000x reference)
"""GraphSAGE (2-layer SAGEConv + log_softmax) on 8 Trainium2 NeuronCores.

Sharding: nodes partitioned contiguously across 8 cores (6250 each), assigned
to 50 tiles of 128 slots per core by a degree-aware bin packing so that every
(tile, src-half) edge set fits in CA=CB=4 blocks of 128 edges (<=512 edges per
tile-half). Tiles are processed in pairs; each pair's gathers are exactly two
1024-index dma_gather instructions (the SWDGE per-instruction fixed cost,
~1us, dominates descriptor generation, so instructions are kept maximal and
aligned with the consuming matmuls).

Math restructure (exact up to fp reassociation):
  l1: agg = segsum_e(x[src_e]); mean = agg * winv[dst]  (winv = 1/max(deg,1))
      h = relu(mean @ Wl1 + b1 + x @ Wr1)
  l2: z = h @ Wl2 ; r = h @ Wr2 + b2   (linear maps pushed before the
      aggregation - valid since segment-mean commutes with them)
      out = log_softmax(segsum_e(z[src_e]) * winv + r)

Aggregation is a one-hot matmul per 128-edge block on TensorE (bf16). The
one-hot is built transposed ([P, dstcol, block]) so the DVE is_equal runs in
2x mode (contiguous 2-byte last dim); the matmuls read strided block slices.
The 1/deg(dst) weight is applied after aggregation. z/r are produced directly
in [node, ch] orientation (lhsT=h, rhs=W2) so no transposes are needed.
dma_gather indices are int16, so gather sources are split in halves (x rows
0..24999 / 25000..49999; z rows 0..25599 / 25600..51199); the x-half of an
edge coincides with its z-half because the z row space is core-major.

HW notes (measured): gather DMA throughput scales ~linearly with the SWDGE
queue count (ucode max 4, all used); single_packet=False is ~1.5x faster for
these 256B-row gathers; num_idxs>1024 per instruction hangs the device (ring
limit); the dense h/zr stages ride almost entirely under the gather stream.
"""
import numpy as np
import ml_dtypes

import concourse.bass as bass
import concourse.bacc as bacc
import concourse.mybir as mybir
import concourse.tile as tile
from concourse import bass_utils

F32 = mybir.dt.float32
BF16 = mybir.dt.bfloat16
I32 = mybir.dt.int32
I16 = mybir.dt.int16
AF = mybir.ActivationFunctionType
OP = mybir.AluOpType
P = 128

# problem constants (hardcoded per contract)
N_NODES = 50000
N_EDGES = 400000
IN_CH = 128
HID = 1024
OUT_CH = 47
NCORES = 8
NPC = N_NODES // NCORES          # nodes per core (6250)
NTILES = 50                      # tiles per core
SLOTS = NTILES * P               # 6400 padded slots per core
NPAIRS = NTILES // 2             # 25 tile pairs
HB = HID // P                    # 8 hid blocks
N_LO = N_NODES // 2              # x gather source split (int16 index range)
Z_LO = (NCORES // 2) * SLOTS     # z gather source split (25600)
NSUB = 8                         # blocks per gather instruction (1024 idx)


def _pair_blocks(ti, CA, CB):
    """Block indices (within a pair's buffer) of tile `ti` (0/1) of the pair.

    Pair block layout: [t0 lo (CA), t1 lo (CA), t0 hi (CB), t1 hi (CB)].
    """
    return ([ti * CA + a for a in range(CA)]
            + [2 * CA + ti * CB + a for a in range(CB)])


def _sp_blocks(ti, sub, npair, CA, CB):
    """Block indices within a superpair gather buffer of tile `ti` of sub-pair
    `sub`. Superpair layout: [pair0 lo (2CA), pair1 lo (2CA), pair0 hi (2CB),
    pair1 hi (2CB)] (lo blocks of all pairs first: each gather instruction
    reads one contiguous slice of the pair-major index stream)."""
    return ([(sub * 2 + ti) * CA + a for a in range(CA)]
            + [npair * 2 * CA + (sub * 2 + ti) * CB + a for a in range(CB)])


GP = 1                           # pairs per gather instruction (1024-idx HW limit)


def build_phase1(CA: int, CB: int, body_reps: int = 1, has_b1: bool = False):
    NPB = 2 * (CA + CB)          # blocks per pair
    NBLK = NPAIRS * NPB
    nc = bacc.Bacc("TRN2", target_bir_lowering=False, debug=False,
                   enable_asserts=False, num_devices=NCORES,
                   num_swdge_queues=4,
                   dynamic_dma_scratch_size=16384 * GP)
    x_lo = nc.dram_tensor("x_lo", [N_LO, IN_CH], BF16, kind="ExternalInput").ap()
    x_hi = nc.dram_tensor("x_hi", [N_NODES - N_LO, IN_CH], BF16, kind="ExternalInput").ap()
    idxA = nc.dram_tensor("idxA", [P, NPAIRS * 2 * CA * 8], I16, kind="ExternalInput").ap()
    idxB = nc.dram_tensor("idxB", [P, NPAIRS * 2 * CB * 8], I16, kind="ExternalInput").ap()
    dstv = nc.dram_tensor("dstv", [P, NBLK], BF16, kind="ExternalInput").ap()
    winvr = nc.dram_tensor("winvr", [P, NTILES, P], BF16, kind="ExternalInput").ap()
    xt_in = nc.dram_tensor("xt_in", [P, SLOTS], BF16, kind="ExternalInput").ap()
    Wl1b = nc.dram_tensor("Wl1b", [P, HID], BF16, kind="ExternalInput").ap()
    Wr1b = nc.dram_tensor("Wr1b", [P, HID], BF16, kind="ExternalInput").ap()
    W2b = nc.dram_tensor("W2b", [P, HB, 2 * OUT_CH], BF16, kind="ExternalInput").ap()
    b1c = nc.dram_tensor("b1c", [P, HB], F32, kind="ExternalInput").ap()
    ones1 = nc.dram_tensor("ones1", [1, 2 * P], BF16, kind="ExternalInput").ap()
    b2rep = nc.dram_tensor("b2rep", [P, OUT_CH], F32, kind="ExternalInput").ap()
    iotar = nc.dram_tensor("iotar", [P, P, NPB], BF16, kind="ExternalInput").ap()

    # z rows are laid out partition-major ([lane, tile, ch]): the phase-2 z-row
    # id of slot (t, q) is q*NTILES + t, so the store is one contiguous DMA
    # per partition instead of 256B row writes.
    z_out = nc.dram_tensor("z_out", [P, NTILES, P], BF16, kind="ExternalOutput").ap()
    r_out = nc.dram_tensor("r_out", [P, NTILES, OUT_CH], F32, kind="ExternalOutput").ap()

    with tile.TileContext(nc) as tc:
        with (
            tc.tile_pool(name="const", bufs=1) as cp,
            tc.tile_pool(name="mgp", bufs=6) as mp,
            tc.tile_pool(name="mgt", bufs=1) as mpt,
            tc.tile_pool(name="ohp", bufs=4) as op_,
            tc.tile_pool(name="work", bufs=3) as wp,
            tc.tile_pool(name="ps_agg", bufs=2, space="PSUM") as psa,
            tc.tile_pool(name="ps_h", bufs=2, space="PSUM") as psh,
            tc.tile_pool(name="ps_z", bufs=2, space="PSUM") as psz,
        ):
            # index/one-hot inputs first: gathers depend only on these
            idxA_sb = cp.tile([P, NPAIRS * 2 * CA * 8], I16)
            nc.sync.dma_start(out=idxA_sb[:], in_=idxA)
            idxB_sb = cp.tile([P, NPAIRS * 2 * CB * 8], I16)
            nc.sync.dma_start(out=idxB_sb[:], in_=idxB)
            dstv_sb = cp.tile([P, NBLK], BF16)
            nc.sync.dma_start(out=dstv_sb[:], in_=dstv)
            iota_sb = cp.tile([P, P, NPB], BF16)
            nc.sync.dma_start(out=iota_sb[:], in_=iotar)
            # big constants last: the first pairs' gathers race them for the
            # DMA engines, and nothing needs these until the first h-matmul
            wl1_sb = cp.tile([P, HID], BF16)
            nc.sync.dma_start(out=wl1_sb[:], in_=Wl1b)
            wr1_sb = cp.tile([P, HID], BF16)
            nc.sync.dma_start(out=wr1_sb[:], in_=Wr1b)
            w2_sb = cp.tile([P, HB, 2 * OUT_CH], BF16)
            nc.sync.dma_start(out=w2_sb[:], in_=W2b)
            b2_sb = cp.tile([P, OUT_CH], F32)
            nc.sync.dma_start(out=b2_sb[:], in_=b2rep)
            # winvr/xt loaded in chunks: first pairs' needs land early via the
            # HWDGE path; the bulk remainder is issued from the gpsimd queue
            # after the first pairs' gathers so it sits behind them in the
            # DMA-engine FIFO
            winv_sb = cp.tile([P, NTILES, P], BF16)
            nc.sync.dma_start(out=winv_sb[:, 0:8, :], in_=winvr[:, 0:8, :])
            xt_sb = cp.tile([P, SLOTS], BF16)
            nc.sync.dma_start(out=xt_sb[:, 0:8 * P], in_=xt_in[:, 0:8 * P])
            nc.sync.dma_start(out=winv_sb[:, 8:NTILES, :],
                              in_=winvr[:, 8:NTILES, :])
            nc.sync.dma_start(out=xt_sb[:, 8 * P:], in_=xt_in[:, 8 * P:])
            if has_b1:
                b1_sb = cp.tile([P, HB], F32)
                nc.sync.dma_start(out=b1_sb[:], in_=b1c)
                b1r_sb = cp.tile([P, HB], BF16)
                nc.vector.tensor_copy(out=b1r_sb[:], in_=b1_sb[:])
                ones_sb = cp.tile([1, 2 * P], BF16)
                nc.sync.dma_start(out=ones_sb[:], in_=ones1)

            z_stage = cp.tile([P, NTILES, P], BF16)
            r_stage = cp.tile([P, NTILES, OUT_CH], F32)

            qi = 0       # round-robin over the 4 SWDGE queues
            for _rep in range(body_reps):
                def _zr_tail(ht, t0):
                    """zr [node128, 94] = h^T (Wl2|Wr2), per 128-node half."""
                    pz = psz.tile([P, 2, 2 * OUT_CH], F32, space="PSUM", tag="psz")
                    for half in range(2):
                        for j in range(HB):
                            nc.tensor.matmul(
                                out=pz[:, half, :],
                                lhsT=ht[:, j, half * P:(half + 1) * P],
                                rhs=w2_sb[:, j, :],
                                start=(j == 0), stop=(j == HB - 1))
                    nc.vector.tensor_copy(out=z_stage[:, t0:t0 + 2, 0:OUT_CH],
                                          in_=pz[:, :, 0:OUT_CH])
                    # r carries the bias: r = h @ Wr2 + b2
                    nc.vector.tensor_tensor(
                        out=r_stage[:, t0:t0 + 2, :],
                        in0=pz[:, :, OUT_CH:2 * OUT_CH],
                        in1=b2_sb[:].rearrange("q (o c) -> q o c", o=1
                                               ).to_broadcast([P, 2, OUT_CH]),
                        op=OP.add)
                    # stream z out every 6 pairs (12 tiles)
                    if t0 % 12 == 10:
                        st = t0 - 10
                        nc.sync.dma_start(out=z_out[:, st:st + 12, :],
                                          in_=z_stage[:, st:st + 12, :])

                prev = None   # (ht, t0) whose zr stage is still pending
                for sp in range(0, NPAIRS, GP):
                    npair = min(GP, NPAIRS - sp)
                    pool = mp if npair == GP else mpt
                    mg = pool.tile([P, npair * NPB, IN_CH], BF16,
                                   tag=f"mg{npair}")
                    ni = npair * 2 * CA * P
                    nc.gpsimd.dma_gather(
                        out_ap=mg[:, 0:npair * 2 * CA, :], in_ap=x_lo,
                        idxs_ap=idxA_sb[:, sp * 2 * CA * 8:(sp + npair) * 2 * CA * 8],
                        num_idxs=ni, num_idxs_reg=ni, elem_size=IN_CH,
                        queue_num=qi % 4, single_packet=False)
                    qi += 1
                    ni = npair * 2 * CB * P
                    nc.gpsimd.dma_gather(
                        out_ap=mg[:, npair * 2 * CA:, :], in_ap=x_hi,
                        idxs_ap=idxB_sb[:, sp * 2 * CB * 8:(sp + npair) * 2 * CB * 8],
                        num_idxs=ni, num_idxs_reg=ni, elem_size=IN_CH,
                        queue_num=qi % 4, single_packet=False)
                    qi += 1
                    for sub in range(npair):
                        p = sp + sub
                        t0 = 2 * p
                        # transposed one-hot [P, dstcol, block]: 2-byte
                        # contiguous last dims keep the DVE in 2x mode
                        oht = op_.tile([P, P, NPB], BF16, tag="oh")
                        nc.vector.tensor_tensor(
                            out=oht[:],
                            in0=dstv_sb[:, p * NPB:(p + 1) * NPB].rearrange(
                                "q (o b) -> q o b", o=1).to_broadcast([P, P, NPB]),
                            in1=iota_sb[:],
                            op=OP.is_equal)

                        ps = psa.tile([P, 2, P], F32, space="PSUM", tag="psagg")
                        for ti in range(2):
                            gblks = _sp_blocks(ti, sub, npair, CA, CB)
                            oblks = _pair_blocks(ti, CA, CB)
                            for i, (gb, ob) in enumerate(zip(gblks, oblks)):
                                nc.tensor.matmul(
                                    out=ps[:, ti, :], lhsT=mg[:, gb, :],
                                    rhs=oht[:, :, ob],
                                    start=(i == 0), stop=(i == len(gblks) - 1))
                        # software pipeline: the previous pair's zr stage
                        # fills the PE while this pair's mean (DVE) and the
                        # previous relu (Act) complete
                        if prev is not None:
                            _zr_tail(*prev)
                        # mean = agg * winv[dst]; cast to bf16
                        mag = wp.tile([P, 2 * P], BF16, tag="mag")
                        nc.vector.tensor_tensor(
                            out=mag[:].rearrange("q (t d) -> q t d", t=2),
                            in0=ps[:],
                            in1=winv_sb[:, t0:t0 + 2, :], op=OP.mult)
                        # hT blocks: [hid128, 256] = Wl1_j^T mean + Wr1_j^T xT
                        ht = wp.tile([P, HB, 2 * P], BF16, tag="ht")
                        for jj in range(2):
                            phh = psh.tile([P, HB // 2, 2 * P], F32,
                                           space="PSUM", tag="psht")
                            for j4 in range(HB // 2):
                                j = jj * (HB // 2) + j4
                                nc.tensor.matmul(
                                    out=phh[:, j4, :],
                                    lhsT=wl1_sb[:, j * P:(j + 1) * P],
                                    rhs=mag[:], start=True, stop=False)
                                if has_b1:
                                    nc.tensor.matmul(out=phh[:, j4, :],
                                                     lhsT=b1r_sb[:, j:j + 1],
                                                     rhs=ones_sb[:],
                                                     start=False, stop=False)
                                nc.tensor.matmul(
                                    out=phh[:, j4, :],
                                    lhsT=wr1_sb[:, j * P:(j + 1) * P],
                                    rhs=xt_sb[:, t0 * P:(t0 + 2) * P],
                                    start=False, stop=True)
                            nc.scalar.activation(
                                out=ht[:, jj * (HB // 2):(jj + 1) * (HB // 2), :],
                                in_=phh[:], func=AF.Relu)
                        prev = (ht, t0)
                _zr_tail(*prev)
                nc.sync.dma_start(out=z_out[:, 48:50, :],
                                  in_=z_stage[:, 48:50, :])
                nc.sync.dma_start(out=r_out, in_=r_stage[:])
    nc.compile()
    return nc


def build_phase2(CA: int, CB: int, body_reps: int = 1):
    NPB = 2 * (CA + CB)
    NBLK = NPAIRS * NPB
    nc = bacc.Bacc("TRN2", target_bir_lowering=False, debug=False,
                   enable_asserts=False, num_devices=NCORES,
                   num_swdge_queues=4,
                   dynamic_dma_scratch_size=16384 * GP)
    z_lo = nc.dram_tensor("z_lo", [Z_LO, P], BF16, kind="ExternalInput").ap()
    z_hi = nc.dram_tensor("z_hi", [NCORES * SLOTS - Z_LO, P], BF16,
                          kind="ExternalInput").ap()
    idxA = nc.dram_tensor("idxA2", [P, NPAIRS * 2 * CA * 8], I16, kind="ExternalInput").ap()
    idxB = nc.dram_tensor("idxB2", [P, NPAIRS * 2 * CB * 8], I16, kind="ExternalInput").ap()
    dstv = nc.dram_tensor("dstv", [P, NBLK], BF16, kind="ExternalInput").ap()
    winv2 = nc.dram_tensor("winv2", [P, NTILES], F32, kind="ExternalInput").ap()
    r_in = nc.dram_tensor("r_in", [P, NTILES, OUT_CH], F32, kind="ExternalInput").ap()
    iotar = nc.dram_tensor("iotar", [P, P, NPB], BF16, kind="ExternalInput").ap()
    out = nc.dram_tensor("out", [P, NTILES, OUT_CH], F32, kind="ExternalOutput").ap()

    with tile.TileContext(nc) as tc:
        with (
            tc.tile_pool(name="const", bufs=1) as cp,
            tc.tile_pool(name="mgp", bufs=6) as mp,
            tc.tile_pool(name="mgt", bufs=1) as mpt,
            tc.tile_pool(name="ohp", bufs=4) as op_,
            tc.tile_pool(name="ps", bufs=2, space="PSUM") as ps,
        ):
            idxA_sb = cp.tile([P, NPAIRS * 2 * CA * 8], I16)
            nc.sync.dma_start(out=idxA_sb[:], in_=idxA)
            idxB_sb = cp.tile([P, NPAIRS * 2 * CB * 8], I16)
            nc.sync.dma_start(out=idxB_sb[:], in_=idxB)
            dstv_sb = cp.tile([P, NBLK], BF16)
            nc.sync.dma_start(out=dstv_sb[:], in_=dstv)
            iota_sb = cp.tile([P, P, NPB], BF16)
            nc.sync.dma_start(out=iota_sb[:], in_=iotar)
            winv_sb = cp.tile([P, NTILES], F32)
            nc.sync.dma_start(out=winv_sb[:], in_=winv2)
            r_all = cp.tile([P, NTILES, OUT_CH], F32)
            nc.sync.dma_start(out=r_all[:], in_=r_in)
            agg = cp.tile([P, NTILES, OUT_CH], F32)
            tsb = cp.tile([P, NTILES, OUT_CH], F32)
            eall = cp.tile([P, NTILES, OUT_CH], F32)
            rmax = cp.tile([P, NTILES, 1], F32)
            esum = cp.tile([P, NTILES, 1], F32)
            lse = cp.tile([P, NTILES, 1], F32)

            def _p2_body(qi=[0]):
                for sp in range(0, NPAIRS, GP):
                    npair = min(GP, NPAIRS - sp)
                    pool = mp if npair == GP else mpt
                    mg = pool.tile([P, npair * NPB, P], BF16, tag=f"m2{npair}")
                    ni = npair * 2 * CA * P
                    nc.gpsimd.dma_gather(
                        out_ap=mg[:, 0:npair * 2 * CA, :], in_ap=z_lo,
                        idxs_ap=idxA_sb[:, sp * 2 * CA * 8:(sp + npair) * 2 * CA * 8],
                        num_idxs=ni, num_idxs_reg=ni, elem_size=P,
                        queue_num=qi[0] % 4, single_packet=False)
                    qi[0] += 1
                    ni = npair * 2 * CB * P
                    nc.gpsimd.dma_gather(
                        out_ap=mg[:, npair * 2 * CA:, :], in_ap=z_hi,
                        idxs_ap=idxB_sb[:, sp * 2 * CB * 8:(sp + npair) * 2 * CB * 8],
                        num_idxs=ni, num_idxs_reg=ni, elem_size=P,
                        queue_num=qi[0] % 4, single_packet=False)
                    qi[0] += 1
                    for sub in range(npair):
                        p = sp + sub
                        _p2_pair(p, sub, npair, mg)

            def _p2_pair(p, sub, npair, mg):
                    t0 = 2 * p
                    oht = op_.tile([P, P, NPB], BF16, tag="oh")
                    nc.vector.tensor_tensor(
                        out=oht[:],
                        in0=dstv_sb[:, p * NPB:(p + 1) * NPB].rearrange(
                            "q (o b) -> q o b", o=1).to_broadcast([P, P, NPB]),
                        in1=iota_sb[:],
                        op=OP.is_equal)
                    # aggregate 8 tiles (4 pairs) per PSUM bank
                    if p % 4 == 0:
                        po = ps.tile([P, 8, OUT_CH], F32, space="PSUM", tag="pso")
                        _p2_pair.po = po
                    po = _p2_pair.po
                    for ti in range(2):
                        gblks = _sp_blocks(ti, sub, npair, CA, CB)
                        oblks = _pair_blocks(ti, CA, CB)
                        for i, (gb, ob) in enumerate(zip(gblks, oblks)):
                            nc.tensor.matmul(out=po[:, (p % 4) * 2 + ti, :],
                                             lhsT=oht[:, :, ob],
                                             rhs=mg[:, gb, 0:OUT_CH],
                                             start=(i == 0),
                                             stop=(i == len(gblks) - 1))
                    if p % 4 == 3 or p == NPAIRS - 1:
                        # incremental epilogue per 8-tile bank (overlaps the
                        # remaining pairs' gathers instead of a serial tail):
                        # t = agg * winv + r ; log_softmax over OUT_CH cols
                        tb = (p // 4) * 8
                        nt = min(8, NTILES - tb)
                        sl = slice(tb, tb + nt)
                        nc.vector.tensor_copy(out=agg[:, sl, :],
                                              in_=po[:, 0:nt, :])
                        nc.vector.tensor_tensor(
                            out=tsb[:, sl, :], in0=agg[:, sl, :],
                            in1=winv_sb[:, sl].rearrange(
                                "q (t o) -> q t o", o=1).to_broadcast(
                                [P, nt, OUT_CH]),
                            op=OP.mult)
                        nc.vector.tensor_tensor(out=tsb[:, sl, :],
                                                in0=tsb[:, sl, :],
                                                in1=r_all[:, sl, :], op=OP.add)
                        nc.vector.tensor_reduce(out=rmax[:, sl, :],
                                                in_=tsb[:, sl, :],
                                                axis=mybir.AxisListType.X,
                                                op=OP.max)
                        nc.vector.tensor_tensor(
                            out=tsb[:, sl, :], in0=tsb[:, sl, :],
                            in1=rmax[:, sl, :].to_broadcast([P, nt, OUT_CH]),
                            op=OP.subtract)
                        nc.scalar.activation(out=eall[:, sl, :],
                                             in_=tsb[:, sl, :], func=AF.Exp)
                        nc.vector.tensor_reduce(out=esum[:, sl, :],
                                                in_=eall[:, sl, :],
                                                axis=mybir.AxisListType.X,
                                                op=OP.add)
                        nc.scalar.activation(out=lse[:, sl, :],
                                             in_=esum[:, sl, :], func=AF.Ln)
                        nc.vector.tensor_tensor(
                            out=eall[:, sl, :], in0=tsb[:, sl, :],
                            in1=lse[:, sl, :].to_broadcast([P, nt, OUT_CH]),
                            op=OP.subtract)
                        nc.sync.dma_start(out=out[:, sl, :],
                                          in_=eall[:, sl, :])

            for _rep in range(body_reps):
                _p2_body()
    nc.compile()
    return nc


def _wrap16(idx_flat: np.ndarray) -> np.ndarray:
    """int16 index stream -> [128, L/16] wrap (16-partition, replicated x8)."""
    L = idx_flat.shape[0]
    w = idx_flat.reshape(L // 16, 16).T.astype(np.int16)
    return np.tile(w, (8, 1))


def _pack_slots(deg_lo, deg_hi):
    """Assign NPC nodes of one core to NTILES tiles (<=128 nodes each) so that
    per-tile lo/hi edge sums stay close to the mean (target <=512 for CA=4).

    Greedy: nodes in decreasing total degree, placed into the feasible tile
    minimizing max(lo_sum, hi_sum) after placement."""
    order = np.argsort(-(deg_lo + deg_hi), kind="stable")
    bin_lo = np.zeros(NTILES)
    bin_hi = np.zeros(NTILES)
    bin_n = np.zeros(NTILES, np.int64)
    assign = np.empty(len(deg_lo), np.int64)
    for i in order:
        score = np.maximum(bin_lo + deg_lo[i], bin_hi + deg_hi[i]) + 1e-3 * bin_n
        score[bin_n >= P] = np.inf
        b = int(np.argmin(score))
        assign[i] = b
        bin_lo[b] += deg_lo[i]
        bin_hi[b] += deg_hi[i]
        bin_n[b] += 1
    # slot within tile = arrival order
    slot = np.empty(len(deg_lo), np.int64)
    cnt = np.zeros(NTILES, np.int64)
    for i in order:
        slot[i] = assign[i] * P + cnt[assign[i]]
        cnt[assign[i]] += 1
    return slot


def _prep(x, edge_index, Wl1, Wr1, b1, Wl2, Wr2, b2):
    """Host-side layout preprocessing (index/layout transforms only)."""
    src = edge_index[0].astype(np.int64)
    dst = edge_index[1].astype(np.int64)
    deg = np.bincount(dst, minlength=N_NODES)
    winv = (1.0 / np.maximum(deg, 1)).astype(np.float32)

    lo_mask = src < N_LO
    deg_lo = np.bincount(dst[lo_mask], minlength=N_NODES)
    deg_hi = deg - deg_lo
    slot_of = np.empty(N_NODES, np.int64)
    for c in range(NCORES):
        nids = np.arange(c * NPC, (c + 1) * NPC)
        slot_of[nids] = _pack_slots(deg_lo[nids], deg_hi[nids])
    core_of = np.minimum(dst // NPC, NCORES - 1)
    # z rows are partition-major within a core: row = lane*NTILES + tile
    zrow = (np.minimum(np.arange(N_NODES) // NPC, NCORES - 1) * SLOTS
            + (slot_of % P) * NTILES + slot_of // P)

    dslot = slot_of[dst]
    dtile = dslot // P
    dlane = dslot % P
    ehalf = (src >= N_LO).astype(np.int64)

    # per (core, tile, half) counts -> CA/CB (global, uniform program)
    key = (core_of * NTILES + dtile) * 2 + ehalf
    counts = np.bincount(key, minlength=NCORES * NTILES * 2).reshape(
        NCORES, NTILES, 2)
    CA = max(int(np.ceil(counts[:, :, 0].max() / P)), 1)
    CB = max(int(np.ceil(counts[:, :, 1].max() / P)), 1)
    NPB = 2 * (CA + CB)
    NBLK = NPAIRS * NPB

    # group edges: sort by (core, tile, half, src)
    order = np.lexsort((src, ehalf, dtile, core_of))
    s_src = src[order]
    s_half = ehalf[order]
    s_tile = dtile[order]
    s_core = core_of[order]
    s_dlane = dlane[order]
    s_zrow = zrow[s_src]

    idx1v = np.zeros((NCORES, NBLK, P), np.int32)
    idx2v = np.zeros((NCORES, NBLK, P), np.int32)
    dstv = np.full((NCORES, NBLK, P), -1.0, np.float32)

    # per-edge destination block/lane, vectorized
    grp = (s_core * NTILES + s_tile) * 2 + s_half    # sorted ascending
    grp_start = np.searchsorted(grp, np.arange(NCORES * NTILES * 2))
    pos = np.arange(len(s_src)) - grp_start[grp]     # rank within group
    pr = s_tile // 2
    ti = s_tile % 2
    base_blk = np.where(
        s_half == 0,
        pr * NPB + ti * CA,
        pr * NPB + 2 * CA + ti * CB)
    blk = base_blk + pos // P
    lane = pos % P
    idx1v[s_core, blk, lane] = (s_src - s_half * N_LO).astype(np.int32)
    idx2v[s_core, blk, lane] = (s_zrow - s_half * Z_LO).astype(np.int32)
    dstv[s_core, blk, lane] = s_dlane.astype(np.float32)

    # gather index streams: pair-major, lo blocks then hi blocks of each pair
    lo_blocks = np.concatenate([
        p * NPB + np.arange(2 * CA) for p in range(NPAIRS)])
    hi_blocks = np.concatenate([
        p * NPB + 2 * CA + np.arange(2 * CB) for p in range(NPAIRS)])

    # transposed-one-hot iota: iotar[q, col, b] = col
    iotar = np.tile(np.arange(P, dtype=np.float32)[None, :, None],
                    (P, 1, NPB)).astype(ml_dtypes.bfloat16)
    b1c = b1.reshape(HB, P).T.astype(np.float32).copy()
    has_b1 = bool(np.abs(b1).max() > 0)
    W2 = np.concatenate([Wl2, Wr2], axis=1).astype(np.float32)  # [HID, 94]
    b2rep = np.tile(b2.astype(np.float32)[None, :], (P, 1))
    ones1 = np.ones((1, 2 * P), np.float32).astype(ml_dtypes.bfloat16)

    wl1b = Wl1.astype(ml_dtypes.bfloat16)
    wr1b = Wr1.astype(ml_dtypes.bfloat16)
    w2b = np.ascontiguousarray(
        W2.reshape(HB, P, 2 * OUT_CH).transpose(1, 0, 2)).astype(ml_dtypes.bfloat16)

    xb = x.astype(ml_dtypes.bfloat16)
    x_lo = np.ascontiguousarray(xb[:N_LO])
    x_hi = np.ascontiguousarray(xb[N_LO:])

    in1_maps, in2_maps = [], []
    for c in range(NCORES):
        nids = np.arange(c * NPC, (c + 1) * NPC)
        xs = np.zeros((SLOTS, IN_CH), np.float32)
        xs[slot_of[nids]] = x[nids]
        xt = np.ascontiguousarray(xs.T).astype(ml_dtypes.bfloat16)  # [128, 6400]
        winv_slot = np.ones(SLOTS, np.float32)
        winv_slot[slot_of[nids]] = winv[nids]
        winvr = np.tile(winv_slot.reshape(1, NTILES, P), (P, 1, 1)).astype(
            ml_dtypes.bfloat16)
        winv2 = np.ascontiguousarray(
            winv_slot.reshape(NTILES, P).T).astype(np.float32)      # [128, 50]

        in1_maps.append({
            "x_lo": x_lo, "x_hi": x_hi,
            "idxA": _wrap16(idx1v[c][lo_blocks].ravel()),
            "idxB": _wrap16(idx1v[c][hi_blocks].ravel()),
            "dstv": np.ascontiguousarray(dstv[c].T).astype(ml_dtypes.bfloat16),
            "winvr": winvr, "xt_in": xt,
            "Wl1b": wl1b, "Wr1b": wr1b, "W2b": w2b,
            "b1c": b1c, "b2rep": b2rep, "ones1": ones1,
            "iotar": iotar,
        })
        in2_maps.append({
            "idxA2": _wrap16(idx2v[c][lo_blocks].ravel()),
            "idxB2": _wrap16(idx2v[c][hi_blocks].ravel()),
            "dstv": np.ascontiguousarray(dstv[c].T).astype(ml_dtypes.bfloat16),
            "winv2": winv2,
            "iotar": iotar,
        })
    return CA, CB, has_b1, in1_maps, in2_maps, slot_of


_cache = {}


def kernel(x, edge_index, Wl1, Wr1, b1, Wl2, Wr2, b2):
    x = np.asarray(x, np.float32)
    edge_index = np.asarray(edge_index)
    CA, CB, has_b1, in1_maps, in2_maps, slot_of = _prep(
        x, edge_index, np.asarray(Wl1, np.float32), np.asarray(Wr1, np.float32),
        np.asarray(b1, np.float32), np.asarray(Wl2, np.float32),
        np.asarray(Wr2, np.float32), np.asarray(b2, np.float32))

    if ("p1", CA, CB, has_b1) not in _cache:
        _cache[("p1", CA, CB, has_b1)] = build_phase1(CA, CB, has_b1=has_b1)
    nc1 = _cache[("p1", CA, CB, has_b1)]
    res1 = bass_utils.run_bass_kernel_spmd(nc1, in1_maps, core_ids=list(range(NCORES)))
    z_all = np.concatenate(
        [res1.results[c]["z_out"].reshape(SLOTS, P) for c in range(NCORES)],
        axis=0)  # [51200, 128] bf16, row = core*SLOTS + lane*NTILES + tile
    for c in range(NCORES):
        in2_maps[c]["z_lo"] = z_all[:Z_LO]
        in2_maps[c]["z_hi"] = z_all[Z_LO:]
        in2_maps[c]["r_in"] = res1.results[c]["r_out"]

    if ("p2", CA, CB) not in _cache:
        _cache[("p2", CA, CB)] = build_phase2(CA, CB)
    nc2 = _cache[("p2", CA, CB)]
    res2 = bass_utils.run_bass_kernel_spmd(nc2, in2_maps, core_ids=list(range(NCORES)))

    out = np.empty((N_NODES, OUT_CH), np.float32)
    for c in range(NCORES):
        o = res2.results[c]["out"]  # [P, NTILES, OUT_CH]
        o = np.ascontiguousarray(o.transpose(1, 0, 2)).reshape(SLOTS, OUT_CH)
        nids = np.arange(c * NPC, (c + 1) * NPC)
        out[nids] = o[slot_of[nids]]
    return out


# ---------------------------------------------------------------------------
# timing utilities. The axon tunnel RTT (~70-90 ms, several-ms jitter) makes
# single-call differential timing useless, so device time is measured by
# slope: dispatch k executions asynchronously in one pipeline (jax dispatch
# is async; block once at the end) and fit (T(k2)-T(k1))/(k2-k1).
# ---------------------------------------------------------------------------

def _make_runner(nc, n_cores):
    import jax
    from jax.sharding import Mesh, PartitionSpec, NamedSharding
    from jax.experimental.shard_map import shard_map
    from concourse import bass2jax

    bass2jax.install_neuronx_cc_hook()
    pname = nc.partition_id_tensor.name if nc.partition_id_tensor else None
    in_names, out_names, out_avals = [], [], []
    for alloc in nc.m.functions[0].allocations:
        if not isinstance(alloc, mybir.MemoryLocationSet):
            continue
        name = alloc.memorylocations[0].name
        if alloc.kind == "ExternalInput":
            if name != pname:
                in_names.append(name)
        elif alloc.kind == "ExternalOutput":
            out_names.append(name)
            out_avals.append(jax.core.ShapedArray(
                tuple(alloc.tensor_shape), mybir.dt.np(alloc.dtype)))
    n_params = len(in_names)
    all_in = list(in_names) + list(out_names)
    if pname is not None:
        all_in.append(pname)

    def _body(*args):
        operands = list(args)
        if pname is not None:
            operands.append(bass2jax.partition_id_tensor())
        outs = bass2jax._bass_exec_p.bind(
            *operands, out_avals=tuple(out_avals), in_names=tuple(all_in),
            out_names=tuple(out_names), lowering_input_output_aliases=(),
            sim_require_finite=False, sim_require_nnan=False, nc=nc)
        return tuple(outs)

    devices = jax.devices()[:n_cores]
    mesh = Mesh(np.asarray(devices), ("core",))
    jitted = jax.jit(
        shard_map(_body, mesh=mesh,
                  in_specs=(PartitionSpec("core"),) * (n_params + len(out_names)),
                  out_specs=(PartitionSpec("core"),) * len(out_names),
                  check_rep=False),
        keep_unused=True)

    def prep(in_maps):
        concat = [np.concatenate([np.asarray(in_maps[c][n]) for c in range(n_cores)], 0)
                  for n in in_names]
        zeros = [np.zeros((n_cores * a.shape[0], *a.shape[1:]), a.dtype)
                 for a in out_avals]
        sh = NamedSharding(mesh, PartitionSpec("core"))
        return [jax.device_put(v, sh) for v in concat + zeros]

    return prep, jitted, out_names


def measure_exec_ns(inp, iters=24, r_lo=6, r_hi=18):
    """Per-execution device time of both phases.

    The host-side dispatch cost through the axon tunnel (~0.5-1 ms per
    execute call, scaling with jit arg count) hides or inflates naive
    per-call timing. Instead each phase is also compiled with its body
    unrolled r_lo x and r_hi x inside one NEFF; the host cost per call is
    identical for both, so device time = (slope(r_hi) - slope(r_lo)) /
    (r_hi - r_lo), where slope(k) is the per-call cost of an async pipeline
    of k calls. This is robust under both serialized and overlapped
    host/device dispatch models."""
    CA, CB, has_b1, in1_maps, in2_maps, slot_of = _prep(
        np.asarray(inp["x"], np.float32), np.asarray(inp["edge_index"]),
        np.asarray(inp["Wl1"], np.float32), np.asarray(inp["Wr1"], np.float32),
        np.asarray(inp["b1"], np.float32), np.asarray(inp["Wl2"], np.float32),
        np.asarray(inp["Wr2"], np.float32), np.asarray(inp["b2"], np.float32))
    for key, builder in (("p1", build_phase1), ("p2", build_phase2)):
        for reps in (1, r_lo, r_hi):
            if (key, CA, CB, reps) not in _cache:
                if key == "p1":
                    _cache[(key, CA, CB, reps)] = builder(CA, CB, body_reps=reps,
                                                          has_b1=has_b1)
                else:
                    _cache[(key, CA, CB, reps)] = builder(CA, CB, body_reps=reps)

    import jax

    prep1, jit1, names1 = _make_runner(_cache[("p1", CA, CB, 1)], NCORES)
    d1 = prep1(in1_maps)
    out1 = jit1(*d1)
    jax.block_until_ready(out1)
    zi = names1.index("z_out")
    ri = names1.index("r_out")
    z_all = np.asarray(out1[zi]).reshape(NCORES * SLOTS, P)  # [core,lane,tile] rows
    for c in range(NCORES):
        in2_maps[c]["z_lo"] = z_all[:Z_LO]
        in2_maps[c]["z_hi"] = z_all[Z_LO:]
        in2_maps[c]["r_in"] = np.asarray(out1[ri]).reshape(
            NCORES, P, NTILES, OUT_CH)[c]

    runners = {}
    for key, maps in (("p1", in1_maps), ("p2", in2_maps)):
        for reps in (r_lo, r_hi):
            prep, jit, _ = _make_runner(_cache[(key, CA, CB, reps)], NCORES)
            runners[(key, reps)] = (jit, prep(maps))

    import time
    K = 12   # pipeline depth per timed burst
    for r in runners.values():
        jax.block_until_ready(r[0](*r[1]))

    def burst(jit, dv):
        t0 = time.perf_counter()
        out = None
        for _ in range(K):
            out = jit(*dv)
        jax.block_until_ready(out)
        return time.perf_counter() - t0

    per_round = {"p1": [], "p2": []}
    for _ in range(iters):
        for key in ("p1", "p2"):
            tlo = burst(*runners[(key, r_lo)])
            thi = burst(*runners[(key, r_hi)])
            per_round[key].append((thi - tlo) / (K * (r_hi - r_lo)) * 1e9)
    p1_ns = max(float(np.median(per_round["p1"])), 0.0)
    p2_ns = max(float(np.median(per_round["p2"])), 0.0)
    p1s = ", ".join(f"{v/1e3:.0f}" for v in per_round["p1"])
    p2s = ", ".join(f"{v/1e3:.0f}" for v in per_round["p2"])
    print(f"  [timing] per-round us estimates p1 [{p1s}] p2 [{p2s}]")
    print(f"  [timing] p1 {p1_ns/1e3:.1f} us/exec, p2 {p2_ns/1e3:.1f} us/exec")
    return int(p1_ns + p2_ns)
